# revision 1
# baseline (speedup 1.0000x reference)
"""GQA kernel for TRN2, 8 NeuronCores.

Sharding: DP2 (batch) x TP4 (head groups). Core r handles batch b=r//4,
quad q=r%4 -> global Q heads 8q..8q+7 (KV groups 2q, 2q+1).

Per-core pipeline (all layouts transposed: [dims, tokens]):
  P1: QKV projections via f32r matmuls from host-pretransposed xT.
      Outputs QT (head-interleaved pairs), KT [Kg0|Kg1, tok], VT -> V' (PE
      transpose, fp16, with ones column for softmax sums).
  P2: scores transposed ST = KT_slice.T @ QT (row-packed pairs g0/g1),
      exp on ACT (no max subtraction; scores ~N(0,1), 1/sqrt(dk) folded
      into Wq on host), PV: O'T = [V|1].T @ exp(ST) accumulated in PSUM.
      Row 64 of O'T = softmax sums; normalize via reciprocal + PE
      broadcast; context staged to DRAM fp16.
  P3: 4-core AllGather of context slices (chunked by token block).
  P4: Y.T slice = Wo_cols.T @ context (fp16), + bo, DMA out.

Host: shards/transposes inputs, assembles [2,2048,2048] output.
"""

import numpy as np

import concourse.bacc as bacc
import concourse.bass as bass
import concourse.mybir as mybir
import concourse.tile as tile
from concourse.bass_utils import run_bass_kernel_spmd
from concourse.masks import make_identity

D = 2048          # d_model
H = 32            # query heads
G = 8             # kv groups
DK = 64           # head dim
B = 2
S = 2048          # tokens per batch
TOK = S
NCORES = 8
NQ = 4            # quads (TP degree)
LH = 8            # local query heads per core
QDIM = LH * DK    # 512 local q dims
KVDIM = 2 * DK    # 128 local kv dims (2 groups)
WCOLS = QDIM + KVDIM + KVDIM  # 768 packed projection cols
NKT = D // 128    # 16 contraction tiles over d_model
NTT = TOK // 128  # 16 token tiles of 128
NC512 = TOK // 512  # 4 token chunks of 512

F32 = mybir.dt.float32
F32R = mybir.dt.float32r
F16 = mybir.dt.float16


def _build_nc() -> bass.Bass:
    nc = bacc.Bacc("TRN2", num_devices=NCORES)

    xt_d = nc.dram_tensor("xt", [D, TOK], F32R, kind="ExternalInput")
    wqkv_d = nc.dram_tensor("wqkv", [D, WCOLS], F32R, kind="ExternalInput")
    bqkv_d = nc.dram_tensor("bqkv", [WCOLS], F32, kind="ExternalInput")
    wo_d = nc.dram_tensor("wo", [D, QDIM], F16, kind="ExternalInput")
    bo_d = nc.dram_tensor("bo", [QDIM], F32, kind="ExternalInput")
    yt_d = nc.dram_tensor("yt", [QDIM, TOK], F32, kind="ExternalOutput")

    # context staging, chunked by 512-token block for chunked AllGather
    ct_src = nc.dram_tensor("ct_src", [NC512, QDIM, 512], F16)
    ct_all = nc.dram_tensor("ct_all", [NC512, D, 512], F16)
    replica_groups = [[0, 1, 2, 3], [4, 5, 6, 7]]

    with tile.TileContext(nc) as tc:
        with tc.tile_pool(name="persist", bufs=1) as persist, \
             tc.tile_pool(name="qt", bufs=1) as qtp, \
             tc.tile_pool(name="ktv", bufs=1) as ktvp:

            # persistent SBUF tensors
            qt = [qtp.tile([128, TOK], F16, tag=f"qt{t}", name=f"qt{t}") for t in range(4)]
            kt = ktvp.tile([128, TOK], F16, tag="kt", name="kt")
            # V' per group: 16 blocks of [128 tok, 64+1]; col 65t+64 is ones
            vp = [ktvp.tile([128, NTT * 65], F16, tag=f"vp{j}", name=f"vp{j}") for j in range(2)]
            bias6 = persist.tile([128, 6], F32)
            bo_t = persist.tile([128, 4], F32)
            ones1 = persist.tile([128, 64], F32R)
            ones_f = persist.tile([128, 64], F32)
            ident = persist.tile([128, 64], F32)

            nc.vector.memset(ones_f[:], 1.0)
            nc.vector.tensor_copy(ones1[:], ones_f[:])
            make_identity(nc, ident[0:64, :])
            make_identity(nc, ident[64:128, :])
            for m in range(6):
                nc.sync.dma_start(bias6[:, m : m + 1],
                                  bqkv_d[bass.ts(m, 128)].unsqueeze(1))
            for m in range(4):
                nc.sync.dma_start(bo_t[:, m : m + 1],
                                  bo_d[bass.ts(m, 128)].unsqueeze(1))

            # ---------------- P1: projections ----------------
            with tc.tile_pool(name="wq", bufs=1) as wpool, \
                 tc.tile_pool(name="xin", bufs=4) as xpool, \
                 tc.tile_pool(name="vt", bufs=1) as vtpool, \
                 tc.tile_pool(name="pproj", bufs=1, space="PSUM") as pproj, \
                 tc.tile_pool(name="ptr", bufs=2, space="PSUM") as ptr:

                w_tiles = []
                for k in range(NKT):
                    wt = wpool.tile([128, WCOLS], F32R, tag=f"w{k}", name=f"w{k}")
                    nc.sync.dma_start(wt[:], wqkv_d[bass.ts(k, 128), :])
                    w_tiles.append(wt)

                vt_sb = vtpool.tile([128, TOK], F32, tag="vt", name="vt")

                for nc5 in range(NC512):
                    ps = [pproj.tile([128, 512], F32, tag=f"p{m}", name=f"p{m}") for m in range(6)]
                    for k in range(NKT):
                        xt_t = xpool.tile([128, 512], F32R, tag="x", name="x")
                        nc.sync.dma_start(
                            xt_t[:], xt_d[bass.ts(k, 128), bass.ts(nc5, 512)])
                        for m in range(6):
                            nc.tensor.matmul(
                                ps[m][:], w_tiles[k][:, bass.ts(m, 128)], xt_t[:],
                                start=(k == 0), stop=(k == NKT - 1))
                    for m in range(4):  # Q -> fp16
                        nc.vector.tensor_scalar_add(
                            qt[m][:, bass.ts(nc5, 512)], ps[m][:], bias6[:, m : m + 1])
                    nc.vector.tensor_scalar_add(  # K -> fp16
                        kt[:, bass.ts(nc5, 512)], ps[4][:], bias6[:, 4:5])
                    nc.vector.tensor_scalar_add(  # V -> f32r staging
                        vt_sb[:, bass.ts(nc5, 512)], ps[5][:], bias6[:, 5:6])

                # V' build: PE transpose [64, 128] -> [128, 64], fp16 + ones col
                for j in range(2):
                    for tt in range(NTT):
                        ps_t = ptr.tile([128, 64], F32, tag="tr", name="tr")
                        nc.tensor.transpose(
                            ps_t[:],
                            vt_sb[bass.ts(j, 64), bass.ts(tt, 128)],
                            ident[bass.ts(j, 64), :])
                        nc.vector.tensor_copy(
                            vp[j][:, bass.ds(tt * 65, 64)], ps_t[:])
                        nc.vector.memset(vp[j][:, bass.ds(tt * 65 + 64, 1)], 1.0)

            # ---------------- P2: attention ----------------
            with tc.tile_pool(name="st", bufs=2) as stp, \
                 tc.tile_pool(name="nrm", bufs=2) as nrmp, \
                 tc.tile_pool(name="ctx", bufs=4) as ctxp, \
                 tc.tile_pool(name="psc", bufs=1, space="PSUM") as psc, \
                 tc.tile_pool(name="pov", bufs=1, space="PSUM") as pov:

                for nc5 in range(NC512):
                    for half in range(2):
                        po = [pov.tile([65, 512], F32, tag=f"o{i}", name=f"o{i}") for i in range(4)]
                        # head slots this half: i2=0..3 -> heads
                        # [2*half, 2*half+4, 2*half+1, 2*half+5]
                        for k in range(NTT):
                            sts = []
                            for i in range(2):
                                t = 2 * half + i
                                s2 = psc.tile([128, 1024], F32, tag=f"s{i}", name=f"s{i}")
                                nc.tensor.matmul(
                                    s2[:, 0:512],
                                    kt[0:64, bass.ts(k, 128)],
                                    qt[t][0:64, bass.ts(nc5, 512)],
                                    start=True, stop=True, tile_position=(0, 0))
                                nc.tensor.matmul(
                                    s2[:, 512:1024],
                                    kt[64:128, bass.ts(k, 128)],
                                    qt[t][64:128, bass.ts(nc5, 512)],
                                    start=True, stop=True, tile_position=(64, 0))
                                sts.append(s2)
                            est = []
                            for i in range(2):
                                e2 = stp.tile([128, 1024], F16, tag=f"e{i}", name=f"e{i}")
                                nc.scalar.activation(
                                    e2[:], sts[i][:], mybir.ActivationFunctionType.Exp)
                                est.append(e2)
                            # slot i2 = 2*i + b2 (b2: 0=g0 rows, 1=g1 rows)
                            for j in range(2):
                                for i in range(2):
                                    i2 = 2 * i + j
                                    nc.tensor.matmul(
                                        po[i2][:], vp[j][:, bass.ds(k * 65, 65)],
                                        est[i][:, bass.ts(j, 512)],
                                        start=(k == 0), stop=(k == NTT - 1))
                        # normalize + stage context
                        for i2 in range(4):
                            i, j = divmod(i2, 2)
                            lhead = 2 * half + i + 4 * j
                            rcp = nrmp.tile([128, 512], F32R, tag="rcp", name="rcp")
                            with nc.allow_low_precision(reason="softmax denom f32r"):
                                nc.vector.reciprocal(rcp[64:65, :], po[i2][64:65, :])
                            bc_ps = psc.tile([64, 512], F32, tag="s0", name=f"bc{i2}")
                            nc.tensor.matmul(
                                bc_ps[:], ones1[64:65, :], rcp[64:65, :],
                                start=True, stop=True, tile_position=(64, 0))
                            bc_sb = nrmp.tile([64, 512], F32, tag="bc", name="bcs")
                            nc.vector.tensor_copy(bc_sb[:], bc_ps[:])
                            ct_t = ctxp.tile([64, 512], F16, tag="ct", name="ct")
                            nc.vector.tensor_mul(
                                ct_t[:], po[i2][0:64, :], bc_sb[:])
                            nc.sync.dma_start(
                                ct_src[nc5, bass.ts(lhead, 64), :], ct_t[:])
                    # chunked AllGather for this token block
                    nc.gpsimd.collective_compute(
                        "AllGather", mybir.AluOpType.bypass,
                        replica_groups=replica_groups,
                        ins=[ct_src[nc5]], outs=[ct_all[nc5]])

            # ---------------- P4: output projection ----------------
            with tc.tile_pool(name="wo", bufs=1) as wop, \
                 tc.tile_pool(name="cin", bufs=4) as cinp, \
                 tc.tile_pool(name="yout", bufs=4) as youtp, \
                 tc.tile_pool(name="py", bufs=1, space="PSUM") as py:

                wo_tiles = []
                for k in range(NKT):
                    wt = wop.tile([128, QDIM], F16, tag=f"wo{k}", name=f"wo{k}")
                    nc.sync.dma_start(wt[:], wo_d[bass.ts(k, 128), :])
                    wo_tiles.append(wt)

                for nc5 in range(NC512):
                    psy = [py.tile([128, 512], F32, tag=f"y{m}", name=f"y{m}") for m in range(4)]
                    for k in range(NKT):
                        ct_t = cinp.tile([128, 512], F16, tag="ci", name="ci")
                        nc.sync.dma_start(ct_t[:], ct_all[nc5, bass.ts(k, 128), :])
                        for m in range(4):
                            nc.tensor.matmul(
                                psy[m][:], wo_tiles[k][:, bass.ts(m, 128)], ct_t[:],
                                start=(k == 0), stop=(k == NKT - 1))
                    for m in range(4):
                        yo = youtp.tile([128, 512], F32, tag="yo", name="yo")
                        nc.vector.tensor_scalar_add(
                            yo[:], psy[m][:], bo_t[:, m : m + 1])
                        nc.sync.dma_start(
                            yt_d[bass.ts(m, 128), bass.ts(nc5, 512)], yo[:])

    nc.compile()
    return nc


_NC_CACHE = None


def _get_nc():
    global _NC_CACHE
    if _NC_CACHE is None:
        _NC_CACHE = _build_nc()
    return _NC_CACHE


def _prep_core_inputs(x, Wq, bq, Wk, bk, Wv, bv, Wo, bo, core):
    b, q = divmod(core, NQ)
    xt = np.ascontiguousarray(x[b].T)  # [D, TOK] f32

    # local head order: pairs (t, t+4) interleaved -> [0,4,1,5,2,6,3,7]
    # local head L (0..7) = global head 8q+L; groups: L0-3 -> g0=2q, L4-7 -> g1=2q+1
    head_order = [0, 4, 1, 5, 2, 6, 3, 7]
    qcols = []
    for L in head_order:
        gh = 8 * q + L
        qcols.extend(range(gh * DK, (gh + 1) * DK))
    kv0 = 2 * q * DK  # global kv dim offset for g0
    kvcols = list(range(kv0, kv0 + 2 * DK))

    wqkv = np.empty((D, WCOLS), dtype=np.float32)
    wqkv[:, :QDIM] = Wq[:, qcols] / 8.0
    wqkv[:, QDIM : QDIM + KVDIM] = Wk[:, kvcols]
    wqkv[:, QDIM + KVDIM :] = Wv[:, kvcols]
    bqkv = np.concatenate([bq[qcols] / 8.0, bk[kvcols], bv[kvcols]]).astype(np.float32)

    out_lo = 512 * q
    wo = np.ascontiguousarray(Wo[:, out_lo : out_lo + QDIM]).astype(np.float16)
    bo_s = np.ascontiguousarray(bo[out_lo : out_lo + QDIM]).astype(np.float32)

    return {"xt": xt, "wqkv": wqkv, "bqkv": bqkv, "wo": wo, "bo": bo_s}


def kernel(x, Wq, bq, Wk, bk, Wv, bv, Wo, bo, _trace=False):
    args = [np.asarray(a, dtype=np.float32)
            for a in (x, Wq, bq, Wk, bk, Wv, bv, Wo, bo)]
    nc = _get_nc()
    in_maps = [_prep_core_inputs(*args, core) for core in range(NCORES)]
    res = run_bass_kernel_spmd(nc, in_maps, core_ids=list(range(NCORES)),
                               trace=_trace)

    y = np.empty((B, S, D), dtype=np.float32)
    for core in range(NCORES):
        b, q = divmod(core, NQ)
        y[b, :, 512 * q : 512 * (q + 1)] = res.results[core]["yt"].T
    if _trace:
        return y, res
    return y

